# revision 8
# baseline (speedup 1.0000x reference)
"""Trainium2 Bass kernel for per-attribute MLP decoder (nn_AttrDecoder).

Computes, for each attribute a (A=312 independent blocks):
    h = relu(x[:, a*64:(a+1)*64] @ W1[a] + b1[a])      # [B, 128]
    o[:, a] = sigmoid(h @ W2[a] + b2[a])               # [B, 1]

Strategy (v2):
  - Data-parallel over batch: B=8192 -> 1024 rows per core across 8 cores.
  - x marshaled on host to bf16, transposed ([A*LAT, B]); DMA'd in 1MB
    chunks (4 attribute-pairs per transfer) to amortize DMA fixed cost.
  - MM1: attributes in pairs; W1[2i] on PE rows 0-63, W1[2i+1] on rows
    64-127; two row-tiled K=64 matmuls run concurrently, N=512 columns,
    h^T accumulated in PSUM.
  - ReLU + b1 fused into the PSUM->SBUF copy (bf16 out), split across
    ScalarE/VectorE by measured throughput (1110 vs 1279 ns per op).
  - MM2 (dense-output): per attr a, stationary is a [128, 32] matrix with
    w2[a] in one column (zeros elsewhere); 32 attrs accumulate into one
    PE col-group's [32, N] PSUM region (start only on the first), four
    col-groups run concurrently.  128 attrs/group land partition-DENSE
    in one [128, 1024] PSUM tile, so sigmoid+b2 is 3 ops/core instead
    of 78 (attr -> partition mapping is unpermuted on the host).
  - Output stored bf16 [384, BS]; host gathers/transposes/casts.
"""

import numpy as np
import ml_dtypes

import concourse.bass as bass
import concourse.tile as tile
from concourse import mybir
from concourse import bass_utils

A = 312
LAT = 64
HID = 128
B = 8192
NCORES = 8
BS = B // NCORES          # 1024 batch rows per core
NPAIR = A // 2            # 156
BT = 512                  # batch tile (one PSUM bank of fp32)
NBT = BS // BT            # 2
XCH = 4                   # attribute-pairs per x DMA (1MB transfers)
NGRP = 3                  # output groups of 128 attrs (128,128,56)
GA = 128                  # attrs per output group

# ACT/DVE relu split: ACT op = 1110ns, DVE op = 1279ns (measured).
# n_act*1.110 + 3.4(sigmoids) == (312-n_act)*1.279  ->  n_act ~ 166
ACT_FRAC = 0.531

_cached = {}


def _attr_slot(a):
    """Partition slot of attr a inside its output group: consecutive attrs
    rotate through the 4 PE col-groups so their MM2 streams overlap."""
    i = a % GA
    return 32 * (i % 4) + i // 4


def _legalize_waits(nc, max_waits=1):
    """Walrus in this toolchain encodes at most one sync-wait per instruction.
    Hoist extra waits onto standalone EventSemaphore instructions placed just
    before the owner on the same engine queue (queue order preserves the
    happens-before)."""
    nsplit = 0
    for bb in nc.m.functions[0].blocks:
        new_insts = []
        changed = False
        for inst in bb.instructions:
            si = getattr(inst, "sync_info", None)
            if si is not None and len(si.on_wait) > max_waits:
                waits = list(si.on_wait)
                for k, w in enumerate(waits[:-max_waits]):
                    es = mybir.InstEventSemaphore(name=f"{inst.name}-hw{k}")
                    es.engine = inst.engine
                    es.opcode = "EventSemaphore"
                    es.sync_info = mybir.SyncInfo(on_wait=[w], on_update=[])
                    new_insts.append(es)
                    nsplit += 1
                inst.sync_info = mybir.SyncInfo(
                    on_wait=waits[-max_waits:], on_update=list(si.on_update))
                changed = True
            new_insts.append(inst)
        if changed:
            bb.instructions = new_insts
    return nsplit


def _build_nc():
    nc = bass.Bass("TRN2", target_bir_lowering=False, debug=False,
                   num_devices=NCORES)
    xt = nc.dram_tensor("xt", [A * LAT, BS], mybir.dt.bfloat16,
                        kind="ExternalInput").ap()
    w1 = nc.dram_tensor("w1", [128, NPAIR, 128], mybir.dt.bfloat16,
                        kind="ExternalInput").ap()
    w2e = nc.dram_tensor("w2e", [HID, A, 32], mybir.dt.bfloat16,
                         kind="ExternalInput").ap()
    b1 = nc.dram_tensor("b1", [HID, A], mybir.dt.float32,
                        kind="ExternalInput").ap()
    b2p = nc.dram_tensor("b2p", [128, 4], mybir.dt.float32,
                         kind="ExternalInput").ap()
    ot = nc.dram_tensor("ot", [NGRP * 128, BS], mybir.dt.bfloat16,
                        kind="ExternalOutput").ap()

    with tile.TileContext(nc, trace_sim=False) as tc:
        _body(tc, xt, w1, w2e, b1, b2p, ot)
    _legalize_waits(nc)
    return nc


def _body(tc, xt, w1, w2e, b1, b2p, ot):
    nc = tc.nc
    from contextlib import ExitStack
    with ExitStack() as ctx:
        singles = ctx.enter_context(tc.tile_pool(name="singles", bufs=1))
        xpool = ctx.enter_context(tc.tile_pool(name="x", bufs=3))
        hpool = ctx.enter_context(tc.tile_pool(name="hsb", bufs=20))
        opool = ctx.enter_context(tc.tile_pool(name="osb", bufs=2))
        hps = ctx.enter_context(
            tc.tile_pool(name="hps", bufs=3, space=bass.MemorySpace.PSUM))
        ops = ctx.enter_context(
            tc.tile_pool(name="ops", bufs=1, space=bass.MemorySpace.PSUM))

        # Resident weights/biases.  Small tensors first (first relu needs
        # b1 immediately); w1/w2e chunked and interleaved so early pairs
        # only wait for their own slice.
        b1_sb = singles.tile([HID, A], mybir.dt.float32)
        nc.gpsimd.dma_start(b1_sb[:], b1[:])
        b2_sb = singles.tile([128, 4], mybir.dt.float32)
        nc.gpsimd.dma_start(b2_sb[:], b2p[:])
        w1_sb = singles.tile([128, NPAIR, 128], mybir.dt.bfloat16)
        w2e_sb = singles.tile([HID, A, 32], mybir.dt.bfloat16)
        # tiny first w1 chunk so pair 0 can start ASAP, then interleave
        # larger w1 / w2e chunks (MM2s start ~7 pairs in, w2e can lag)
        nc.gpsimd.dma_start(w1_sb[:, 0:4, :], w1[:, 0:4, :])
        # warm the ACT sigmoid/relu table set during the DMA ramp
        scratch = singles.tile([128, 1], mybir.dt.float32)
        nc.scalar.activation(out=scratch[:], in_=b2_sb[:, 3:4],
                             func=mybir.ActivationFunctionType.Sigmoid,
                             bias=b2_sb[:, 3:4], scale=1.0)
        w1_chunks = [(4, 24), (24, 48), (48, 96), (96, 156)]
        w2e_chunks = [(0, 78), (78, 156), (156, 234), (234, 312)]
        for (p0, p1), (a0, a1) in zip(w1_chunks, w2e_chunks):
            nc.gpsimd.dma_start(w1_sb[:, p0:p1, :], w1[:, p0:p1, :])
            nc.gpsimd.dma_start(w2e_sb[:, a0:a1, :], w2e[:, a0:a1, :])

        def w1_slice(p, j):
            return w1_sb[j * 64:(j + 1) * 64, p, :]

        act_acc = [0.0]

        def emit_relu(a, h_ps):
            """PSUM->SBUF evacuation with bias+relu, engine chosen to
            balance measured ACT(1110ns)/DVE(1279ns) op costs."""
            h_sb = hpool.tile([HID, NBT, BT], mybir.dt.bfloat16, name="hsb")
            act_acc[0] += ACT_FRAC
            if act_acc[0] >= 1.0:
                act_acc[0] -= 1.0
                nc.scalar.activation(
                    out=h_sb[:], in_=h_ps[:],
                    func=mybir.ActivationFunctionType.Relu,
                    bias=b1_sb[:, a:a + 1], scale=1.0)
            else:
                nc.vector.tensor_scalar(
                    out=h_sb[:], in0=h_ps[:],
                    scalar1=b1_sb[:, a:a + 1], scalar2=0.0,
                    op0=mybir.AluOpType.add,
                    op1=mybir.AluOpType.max)
            return h_sb

        def emit_mm2_quad(quad, o_ps):
            """One quad (4 consecutive attrs = 4 distinct PE col-groups):
            8 accumulating M=32 matmuls, bt-outer so the 4 streams hit
            4 different col-groups back-to-back (concurrent)."""
            for bt in range(NBT):
                for a, h_sb in quad:
                    i = a % GA
                    j = i % 4          # col group
                    r = i // 4         # column inside the stationary
                    glast = min(A - 1, (a // GA) * GA + GA - 1) % GA // 4
                    nc.tensor.matmul(
                        o_ps[32 * j:32 * j + 32, bt, :],
                        w2e_sb[:, a, :],
                        h_sb[:, bt, :],
                        start=(r == 0), stop=(r == glast),
                        tile_position=(0, 32 * j),
                    )

        def drain_group(g, o_ps):
            """Sigmoid + b2 over the dense [128, BS] group output, then
            one contiguous bf16 store."""
            o_sb = opool.tile([128, NBT, BT], mybir.dt.bfloat16, name="osb")
            nc.scalar.activation(
                out=o_sb[:], in_=o_ps[:],
                func=mybir.ActivationFunctionType.Sigmoid,
                bias=b2_sb[:, g:g + 1], scale=1.0)
            nc.sync.dma_start(
                out=ot[g * 128:(g + 1) * 128, :].rearrange(
                    "p (n b) -> p n b", n=NBT),
                in_=o_sb[:])

        pend = []               # (a, h_sb) not yet MM2'd
        o_ps = None
        DEFER = 14              # emit a quad only once its relus are ~5
                                # pairs old, so MM2s never stall the PE queue
        # x chunk schedule: small first chunks so pair 0 starts ASAP
        chunks = [1, 3] + [XCH] * ((NPAIR - 4) // XCH)
        chunk_starts = [0]
        for n in chunks[:-1]:
            chunk_starts.append(chunk_starts[-1] + n)
        next_chunk = 0
        x_tile = None
        q0 = 0
        for p in range(NPAIR):
            if next_chunk < len(chunks) and p == chunk_starts[next_chunk]:
                n = chunks[next_chunk]
                x_tile = xpool.tile([128, XCH, BS], mybir.dt.bfloat16)
                nc.sync.dma_start(
                    out=x_tile[:, 0:n, :],
                    in_=xt[p * 128:(p + n) * 128, :].rearrange(
                        "(q c) b -> c q b", q=n))
                q0 = p
                next_chunk += 1
            q = p - q0
            if len(pend) >= DEFER:
                emit_mm2_quad(pend[:4], o_ps)
                pend = pend[4:]
            # attr-major: each attr's two MM1s then its relu immediately,
            # so the PSUM tile is produced->consumed with minimum latency.
            for j in range(2):
                a = 2 * p + j
                if a % GA == 0:     # new output group: fresh PSUM bank pair
                    if o_ps is not None and pend:
                        # flush the previous group's trailing quads first
                        while pend:
                            emit_mm2_quad(pend[:4], o_ps)
                            pend = pend[4:]
                    if a > 0:
                        drain_group(a // GA - 1, o_ps)
                    o_ps = ops.tile([128, NBT, BT], mybir.dt.float32,
                                    name="o_dense")
                h_ps = hps.tile([128, NBT, BT], mybir.dt.float32, name="hps")
                for bt in range(NBT):
                    nc.tensor.matmul(
                        h_ps[:, bt, :],
                        w1_slice(p, j),
                        x_tile[j * 64:(j + 1) * 64, q, bass.ds(bt * BT, BT)],
                        start=True, stop=True,
                        tile_position=(j * 64, 0),
                    )
                pend.append((a, emit_relu(a, h_ps)))
        while pend:
            emit_mm2_quad(pend[:4], o_ps)
            pend = pend[4:]
        drain_group(NGRP - 1, o_ps)


def _install_ntff_hook():
    """Register the axon NTFF profile hook (normally provided by the agent
    image's antenv.axon_hooks). Needed only for trace=True runs."""
    import sys as _sys, types as _types, ctypes, contextlib

    if "antenv.axon_hooks" not in _sys.modules:
        mod = _types.ModuleType("antenv.axon_hooks")
        _h = [None]
        mod.set_axon_ntff_profile_hook = lambda h: _h.__setitem__(0, h)
        mod.get_axon_ntff_profile_hook = lambda: _h[0]
        _sys.modules["antenv.axon_hooks"] = mod
        try:
            import antenv
            antenv.axon_hooks = mod
        except ImportError:
            pass
    mod = _sys.modules["antenv.axon_hooks"]
    if mod.get_axon_ntff_profile_hook() is not None:
        return

    lib = ctypes.CDLL("/opt/axon/libaxon_pjrt.so")
    lib.axon_start_nrt_profile.argtypes = [
        ctypes.POINTER(ctypes.c_int64), ctypes.c_size_t]
    lib.axon_start_nrt_profile.restype = ctypes.c_int64
    lib.axon_stop_nrt_profile.argtypes = [ctypes.c_char_p]
    lib.axon_stop_nrt_profile.restype = ctypes.c_int64

    @contextlib.contextmanager
    def _hook(output_dir, device_ids):
        import jax
        jax.devices()
        if device_ids:
            ids = (ctypes.c_int64 * len(device_ids))(*device_ids)
            rc = lib.axon_start_nrt_profile(ids, len(device_ids))
        else:
            rc = lib.axon_start_nrt_profile(None, 0)
        if rc != 0:
            raise RuntimeError(f"axon_start_nrt_profile rc={rc}")
        try:
            yield
        finally:
            n = lib.axon_stop_nrt_profile(str(output_dir).encode())
            print(f"ntff profile: {n} file(s) -> {output_dir}")

    mod.set_axon_ntff_profile_hook(_hook)
    # artifact upload needs a bucket; stub it out for local profiling
    bass_utils.upload_artifacts = lambda tmpdir: f"local://{tmpdir}"


def kernel(x, W1, b1, W2, b2, trace=False):
    if "nc" not in _cached:
        _cached["nc"] = _build_nc()
    nc = _cached["nc"]
    if trace:
        try:
            _install_ntff_hook()
        except Exception as e:
            print("ntff hook install failed:", e)
            trace = False

    xt = np.ascontiguousarray(
        x.reshape(B, A * LAT).astype(ml_dtypes.bfloat16).T)     # [19968, 8192]
    w1h = np.ascontiguousarray(
        W1.reshape(NPAIR, 128, 128).transpose(1, 0, 2)).astype(
            ml_dtypes.bfloat16)                                  # [128,156,128]
    # expanded W2: attr a's w2 sits in column slot(a)//... (col r = (a%128)//4)
    w2eh = np.zeros((HID, A, 32), np.float32)
    for a in range(A):
        w2eh[:, a, (a % GA) // 4] = W2[a, :, 0]
    w2eh = w2eh.astype(ml_dtypes.bfloat16)
    b1h = np.ascontiguousarray(b1.T).astype(np.float32)          # [128, 312]
    b2ph = np.zeros((128, 4), np.float32)
    slots = np.array([_attr_slot(a) for a in range(A)])
    b2ph[slots, np.arange(A) // GA] = b2[:, 0]

    in_maps = []
    for c in range(NCORES):
        in_maps.append({
            "xt": np.ascontiguousarray(xt[:, c * BS:(c + 1) * BS]),
            "w1": w1h, "w2e": w2eh, "b1": b1h, "b2p": b2ph,
        })

    res = bass_utils.run_bass_kernel_spmd(
        nc, in_maps, core_ids=list(range(NCORES)), trace=trace)
    _cached["last_results"] = res

    rows = (np.arange(A) // GA) * 128 + slots                    # [312]
    out = np.empty((B, A), np.float32)
    for c in range(NCORES):
        ot_c = np.asarray(res.results[c]["ot"])                  # [384, BS] bf16
        out[c * BS:(c + 1) * BS, :] = ot_c[rows, :].T.astype(np.float32)
    return out


# revision 13
# speedup vs baseline: 1.0515x; 1.0515x over previous
"""Trainium2 Bass kernel for per-attribute MLP decoder (nn_AttrDecoder).

Computes, for each attribute a (A=312 independent blocks):
    h = relu(x[:, a*64:(a+1)*64] @ W1[a] + b1[a])      # [B, 128]
    o[:, a] = sigmoid(h @ W2[a] + b2[a])               # [B, 1]

Strategy (v2):
  - Data-parallel over batch: B=8192 -> 1024 rows per core across 8 cores.
  - x marshaled on host to bf16, transposed ([A*LAT, B]); DMA'd in 1MB
    chunks (4 attribute-pairs per transfer) to amortize DMA fixed cost.
  - MM1: attributes in pairs; W1[2i] on PE rows 0-63, W1[2i+1] on rows
    64-127; two row-tiled K=64 matmuls run concurrently, N=512 columns,
    h^T accumulated in PSUM.
  - ReLU + b1 fused into the PSUM->SBUF copy (bf16 out), split across
    ScalarE/VectorE by measured throughput (1110 vs 1279 ns per op).
  - MM2 (dense-output): per attr a, stationary is a [128, 32] matrix with
    w2[a] in one column (zeros elsewhere); 32 attrs accumulate into one
    PE col-group's [32, N] PSUM region (start only on the first), four
    col-groups run concurrently.  128 attrs/group land partition-DENSE
    in one [128, 1024] PSUM tile, so sigmoid+b2 is 3 ops/core instead
    of 78 (attr -> partition mapping is unpermuted on the host).
  - Output stored bf16 [384, BS]; host gathers/transposes/casts.
"""

import numpy as np
import ml_dtypes

import concourse.bass as bass
import concourse.tile as tile
from concourse import mybir
from concourse import bass_utils

A = 312
LAT = 64
HID = 128
B = 8192
NCORES = 8
BS = B // NCORES          # 1024 batch rows per core
NPAIR = A // 2            # 156
BT = 512                  # batch tile (one PSUM bank of fp32)
NBT = BS // BT            # 2
XCH = 4                   # attribute-pairs per x DMA (1MB transfers)
NGRP = 3                  # output groups of 128 attrs (128,128,56)
GA = 128                  # attrs per output group

# ACT/DVE relu split: ACT op = 1110ns, DVE op = 1279ns (measured).
# n_act*1.110 + 3.4(sigmoids) == (312-n_act)*1.279  ->  n_act ~ 166
ACT_FRAC = 0.531

_cached = {}


def _attr_slot(a):
    """Partition slot of attr a inside its output group: consecutive attrs
    rotate through the 4 PE col-groups so their MM2 streams overlap."""
    i = a % GA
    return 32 * (i % 4) + i // 4


def _legalize_waits(nc, max_waits=1):
    """Walrus in this toolchain encodes at most one sync-wait per instruction.
    Hoist extra waits onto standalone EventSemaphore instructions placed just
    before the owner on the same engine queue (queue order preserves the
    happens-before)."""
    nsplit = 0
    for bb in nc.m.functions[0].blocks:
        new_insts = []
        changed = False
        for inst in bb.instructions:
            si = getattr(inst, "sync_info", None)
            if si is not None and len(si.on_wait) > max_waits:
                waits = list(si.on_wait)
                for k, w in enumerate(waits[:-max_waits]):
                    es = mybir.InstEventSemaphore(name=f"{inst.name}-hw{k}")
                    es.engine = inst.engine
                    es.opcode = "EventSemaphore"
                    es.sync_info = mybir.SyncInfo(on_wait=[w], on_update=[])
                    new_insts.append(es)
                    nsplit += 1
                inst.sync_info = mybir.SyncInfo(
                    on_wait=waits[-max_waits:], on_update=list(si.on_update))
                changed = True
            new_insts.append(inst)
        if changed:
            bb.instructions = new_insts
    return nsplit


def _build_nc():
    nc = bass.Bass("TRN2", target_bir_lowering=False, debug=False,
                   num_devices=NCORES)
    xt = nc.dram_tensor("xt", [A * LAT, BS], mybir.dt.bfloat16,
                        kind="ExternalInput").ap()
    w1 = nc.dram_tensor("w1", [128, NPAIR, 128], mybir.dt.bfloat16,
                        kind="ExternalInput").ap()
    w2e = nc.dram_tensor("w2e", [HID, A, 32], mybir.dt.bfloat16,
                         kind="ExternalInput").ap()
    b1 = nc.dram_tensor("b1", [HID, A], mybir.dt.float32,
                        kind="ExternalInput").ap()
    b2p = nc.dram_tensor("b2p", [128, 4], mybir.dt.float32,
                         kind="ExternalInput").ap()
    ot = nc.dram_tensor("ot", [NGRP * 128, BS], mybir.dt.bfloat16,
                        kind="ExternalOutput").ap()

    with tile.TileContext(nc, trace_sim=False) as tc:
        _body(tc, xt, w1, w2e, b1, b2p, ot)
    _legalize_waits(nc)
    return nc


def _body(tc, xt, w1, w2e, b1, b2p, ot):
    nc = tc.nc
    from contextlib import ExitStack
    with ExitStack() as ctx:
        singles = ctx.enter_context(tc.tile_pool(name="singles", bufs=1))
        xpool = ctx.enter_context(tc.tile_pool(name="x", bufs=3))
        hpool = ctx.enter_context(tc.tile_pool(name="hsb", bufs=20))
        opool = ctx.enter_context(tc.tile_pool(name="osb", bufs=2))
        hps = ctx.enter_context(
            tc.tile_pool(name="hps", bufs=3, space=bass.MemorySpace.PSUM))
        ops = ctx.enter_context(
            tc.tile_pool(name="ops", bufs=1, space=bass.MemorySpace.PSUM))

        # Resident weights/biases.  Small tensors first (first relu needs
        # b1 immediately); w1/w2e chunked and interleaved so early pairs
        # only wait for their own slice.
        b1_sb = singles.tile([HID, A], mybir.dt.float32)
        nc.gpsimd.dma_start(b1_sb[:], b1[:])
        b2_sb = singles.tile([128, 4], mybir.dt.float32)
        nc.gpsimd.dma_start(b2_sb[:], b2p[:])
        w1_sb = singles.tile([128, NPAIR, 128], mybir.dt.bfloat16)
        w2e_sb = singles.tile([HID, A, 32], mybir.dt.bfloat16)
        # tiny first w1 chunk so pair 0 can start ASAP, then interleave
        # larger w1 / w2e chunks (MM2s start ~7 pairs in, w2e can lag)
        nc.gpsimd.dma_start(w1_sb[:, 0:4, :], w1[:, 0:4, :])
        # warm the ACT sigmoid/relu table set during the DMA ramp
        scratch = singles.tile([128, 1], mybir.dt.float32)
        nc.scalar.activation(out=scratch[:], in_=b2_sb[:, 3:4],
                             func=mybir.ActivationFunctionType.Sigmoid,
                             bias=b2_sb[:, 3:4], scale=1.0)
        w1_chunks = [(4, 24), (24, 48), (48, 96), (96, 156)]
        w2e_chunks = [(0, 78), (78, 156), (156, 234), (234, 312)]
        for (p0, p1), (a0, a1) in zip(w1_chunks, w2e_chunks):
            nc.gpsimd.dma_start(w1_sb[:, p0:p1, :], w1[:, p0:p1, :])
            nc.gpsimd.dma_start(w2e_sb[:, a0:a1, :], w2e[:, a0:a1, :])

        def w1_slice(p, j):
            return w1_sb[j * 64:(j + 1) * 64, p, :]

        # PE warmup: ~24 back-to-back garbage matmuls during the DMA ramp
        # flip the HAM clock gate to K=8/8 (2.4 GHz) before real work;
        # without this the first ~45us of matmuls run at 1.2 GHz.  They
        # scribble on group 0's o_ps tile, which is safe: every col-group's
        # first real MM2 has start=True, clearing the bank.
        garbage = singles.tile([128, 640], mybir.dt.bfloat16)
        nc.gpsimd.memset(garbage[:], 0.0)
        o_ps0 = ops.tile([128, NBT, BT], mybir.dt.float32, name="o_dense")
        for _ in range(24):
            nc.tensor.matmul(
                o_ps0[:, 0, :], garbage[:, 0:128], garbage[:, 128:640],
                start=True, stop=True)

        act_acc = [0.0]

        def emit_relu(a, h_ps):
            """PSUM->SBUF evacuation with bias+relu, engine chosen to
            balance measured ACT(1110ns)/DVE(1279ns) op costs."""
            h_sb = hpool.tile([HID, NBT, BT], mybir.dt.bfloat16, name="hsb")
            act_acc[0] += ACT_FRAC
            if act_acc[0] >= 1.0:
                act_acc[0] -= 1.0
                nc.scalar.activation(
                    out=h_sb[:], in_=h_ps[:],
                    func=mybir.ActivationFunctionType.Relu,
                    bias=b1_sb[:, a:a + 1], scale=1.0)
            else:
                nc.vector.tensor_scalar(
                    out=h_sb[:], in0=h_ps[:],
                    scalar1=b1_sb[:, a:a + 1], scalar2=0.0,
                    op0=mybir.AluOpType.add,
                    op1=mybir.AluOpType.max)
            return h_sb

        def emit_mm2_quad(quad, o_ps):
            """One quad (4 consecutive attrs = 4 distinct PE col-groups):
            8 accumulating M=32 matmuls, bt-outer so the 4 streams hit
            4 different col-groups back-to-back (concurrent)."""
            for bt in range(NBT):
                for a, h_sb in quad:
                    i = a % GA
                    j = i % 4          # col group
                    r = i // 4         # column inside the stationary
                    glast = min(A - 1, (a // GA) * GA + GA - 1) % GA // 4
                    nc.tensor.matmul(
                        o_ps[32 * j:32 * j + 32, bt, :],
                        w2e_sb[:, a, :],
                        h_sb[:, bt, :],
                        start=(r == 0), stop=(r == glast),
                        tile_position=(0, 32 * j),
                    )

        def drain_group(g, o_ps):
            """Sigmoid + b2 over the dense [128, BS] group output, then
            one contiguous bf16 store."""
            o_sb = opool.tile([128, NBT, BT], mybir.dt.bfloat16, name="osb")
            nc.scalar.activation(
                out=o_sb[:], in_=o_ps[:],
                func=mybir.ActivationFunctionType.Sigmoid,
                bias=b2_sb[:, g:g + 1], scale=1.0)
            nc.sync.dma_start(
                out=ot[g * 128:(g + 1) * 128, :].rearrange(
                    "p (n b) -> p n b", n=NBT),
                in_=o_sb[:])

        pend = []               # (a, h_sb) not yet MM2'd
        o_ps = None
        DEFER = 14              # emit a quad only once its relus are ~5
                                # pairs old, so MM2s never stall the PE queue
        # x chunk schedule: small first chunks so pair 0 starts ASAP
        chunks = [1, 3] + [XCH] * ((NPAIR - 4) // XCH)
        chunk_starts = [0]
        for n in chunks[:-1]:
            chunk_starts.append(chunk_starts[-1] + n)
        next_chunk = 0
        x_tile = None
        q0 = 0
        for p in range(NPAIR):
            if next_chunk < len(chunks) and p == chunk_starts[next_chunk]:
                n = chunks[next_chunk]
                x_tile = xpool.tile([128, XCH, BS], mybir.dt.bfloat16)
                nc.sync.dma_start(
                    out=x_tile[:, 0:n, :],
                    in_=xt[p * 128:(p + n) * 128, :].rearrange(
                        "(q c) b -> c q b", q=n))
                q0 = p
                next_chunk += 1
            q = p - q0
            if len(pend) >= DEFER:
                emit_mm2_quad(pend[:4], o_ps)
                pend = pend[4:]
            if (2 * p) % GA == 0:   # new output group: fresh PSUM bank pair
                if p == 0:
                    o_ps = o_ps0
                else:
                    # flush the previous group's trailing quads first
                    while pend:
                        emit_mm2_quad(pend[:4], o_ps)
                        pend = pend[4:]
                    drain_group((2 * p) // GA - 1, o_ps)
                    o_ps = ops.tile([128, NBT, BT], mybir.dt.float32,
                                    name="o_dense")
            h_pss = [hps.tile([128, NBT, BT], mybir.dt.float32, name="hps"),
                     hps.tile([128, NBT, BT], mybir.dt.float32, name="hps")]
            # MM1: the two attrs of the pair on disjoint PE row halves,
            # bt-major so the two row-tiled streams run concurrently.
            for bt in range(NBT):
                for j in range(2):
                    nc.tensor.matmul(
                        h_pss[j][:, bt, :],
                        w1_slice(p, j),
                        x_tile[j * 64:(j + 1) * 64, q, bass.ds(bt * BT, BT)],
                        start=True, stop=True,
                        tile_position=(j * 64, 0),
                    )
            for j in range(2):
                pend.append((2 * p + j, emit_relu(2 * p + j, h_pss[j])))
        while pend:
            emit_mm2_quad(pend[:4], o_ps)
            pend = pend[4:]
        drain_group(NGRP - 1, o_ps)


def _install_ntff_hook():
    """Register the axon NTFF profile hook (normally provided by the agent
    image's antenv.axon_hooks). Needed only for trace=True runs."""
    import sys as _sys, types as _types, ctypes, contextlib

    if "antenv.axon_hooks" not in _sys.modules:
        mod = _types.ModuleType("antenv.axon_hooks")
        _h = [None]
        mod.set_axon_ntff_profile_hook = lambda h: _h.__setitem__(0, h)
        mod.get_axon_ntff_profile_hook = lambda: _h[0]
        _sys.modules["antenv.axon_hooks"] = mod
        try:
            import antenv
            antenv.axon_hooks = mod
        except ImportError:
            pass
    mod = _sys.modules["antenv.axon_hooks"]
    if mod.get_axon_ntff_profile_hook() is not None:
        return

    lib = ctypes.CDLL("/opt/axon/libaxon_pjrt.so")
    lib.axon_start_nrt_profile.argtypes = [
        ctypes.POINTER(ctypes.c_int64), ctypes.c_size_t]
    lib.axon_start_nrt_profile.restype = ctypes.c_int64
    lib.axon_stop_nrt_profile.argtypes = [ctypes.c_char_p]
    lib.axon_stop_nrt_profile.restype = ctypes.c_int64

    @contextlib.contextmanager
    def _hook(output_dir, device_ids):
        import jax
        jax.devices()
        if device_ids:
            ids = (ctypes.c_int64 * len(device_ids))(*device_ids)
            rc = lib.axon_start_nrt_profile(ids, len(device_ids))
        else:
            rc = lib.axon_start_nrt_profile(None, 0)
        if rc != 0:
            raise RuntimeError(f"axon_start_nrt_profile rc={rc}")
        try:
            yield
        finally:
            n = lib.axon_stop_nrt_profile(str(output_dir).encode())
            print(f"ntff profile: {n} file(s) -> {output_dir}")

    mod.set_axon_ntff_profile_hook(_hook)
    # artifact upload needs a bucket; stub it out for local profiling
    bass_utils.upload_artifacts = lambda tmpdir: f"local://{tmpdir}"


def kernel(x, W1, b1, W2, b2, trace=False):
    if "nc" not in _cached:
        _cached["nc"] = _build_nc()
    nc = _cached["nc"]
    if trace:
        try:
            _install_ntff_hook()
        except Exception as e:
            print("ntff hook install failed:", e)
            trace = False

    xt = np.ascontiguousarray(
        x.reshape(B, A * LAT).astype(ml_dtypes.bfloat16).T)     # [19968, 8192]
    w1h = np.ascontiguousarray(
        W1.reshape(NPAIR, 128, 128).transpose(1, 0, 2)).astype(
            ml_dtypes.bfloat16)                                  # [128,156,128]
    # expanded W2: attr a's w2 sits in column slot(a)//... (col r = (a%128)//4)
    w2eh = np.zeros((HID, A, 32), np.float32)
    for a in range(A):
        w2eh[:, a, (a % GA) // 4] = W2[a, :, 0]
    w2eh = w2eh.astype(ml_dtypes.bfloat16)
    b1h = np.ascontiguousarray(b1.T).astype(np.float32)          # [128, 312]
    b2ph = np.zeros((128, 4), np.float32)
    slots = np.array([_attr_slot(a) for a in range(A)])
    b2ph[slots, np.arange(A) // GA] = b2[:, 0]

    in_maps = []
    for c in range(NCORES):
        in_maps.append({
            "xt": np.ascontiguousarray(xt[:, c * BS:(c + 1) * BS]),
            "w1": w1h, "w2e": w2eh, "b1": b1h, "b2p": b2ph,
        })

    res = bass_utils.run_bass_kernel_spmd(
        nc, in_maps, core_ids=list(range(NCORES)), trace=trace)
    _cached["last_results"] = res

    rows = (np.arange(A) // GA) * 128 + slots                    # [312]
    out = np.empty((B, A), np.float32)
    for c in range(NCORES):
        ot_c = np.asarray(res.results[c]["ot"])                  # [384, BS] bf16
        out[c * BS:(c + 1) * BS, :] = ot_c[rows, :].T.astype(np.float32)
    return out
